# revision 7
# baseline (speedup 1.0000x reference)
"""Trainium2 Bass kernel for a single-step attention decoder RNN (AttnDecoderRNN).

Math (reference semantics, N=64 batch, S=256 src len, H=1024 hidden):
  GRU step (PyTorch gate order r,z,n) -> h_new
  energy = enc @ W_attn.T + b_attn ; scores = einsum('nh,nsh->ns', h_new, energy)
  attn = softmax(scores) ; context = einsum('ns,nsh->nh', attn, enc)
  output = cat(h_new, context) @ W_out.T + b_out

Algebraic restructure: scores[n,s] = enc[n,s,:] . q[n,:] + h_new[n,:].b_attn with
q = h_new @ W_attn. The per-row constant h_new.b_attn cancels in softmax, so the
[N,S,H] energy tensor is never materialized (34 GFLOP -> 1.7 GFLOP).

Distribution over 8 NeuronCores:
  - GRU tensor-parallel over the hidden dim: core c computes gate columns
    [128c:128c+128) of r,z,n for ALL 64 batches (weights pre-sharded on host).
  - An AllToAll exchanges [128-hidden-slice x 8-batch-block] tiles so each core
    ends up with rnn^T [1024, 8] for ITS 8 batches (no dynamic indexing).
  - Attention + output GEMM are batch-parallel (8 batches/core).
Matmuls run as float32r (reduced-precision fp32, 1 cyc/row, ~1.6e-4 rel err);
the context/output GEMMs use bf16 operands (their outputs tolerate ~2e-3).
"""
import numpy as np
import ml_dtypes
from contextlib import ExitStack

N, S, H = 64, 256, 1024
NCORES = 8
NB = N // NCORES        # 8 batches per core
HS = H // NCORES        # 128 hidden columns per core
KX = 2 * H // 128       # 16 contraction chunks for W_ih
KH = H // 128           # 8 contraction chunks for W_hh / W_attn
BF16 = ml_dtypes.bfloat16

_cache = {}
_last_results = None


def _build():
    import concourse.tile as tile
    from concourse import bacc, mybir

    f32 = mybir.dt.float32
    f32r = mybir.dt.float32r
    bf16 = mybir.dt.bfloat16

    nc = bacc.Bacc("TRN2", target_bir_lowering=False, debug=False,
                   enable_asserts=False, num_devices=NCORES)

    xT = nc.dram_tensor("xT", [2 * H, N], f32r, kind="ExternalInput").ap()
    hT = nc.dram_tensor("hT", [H, N], f32r, kind="ExternalInput").ap()
    hsl = nc.dram_tensor("hsl", [N, HS], f32, kind="ExternalInput").ap()
    wih = nc.dram_tensor("wih", [2 * H, 3 * HS], f32r, kind="ExternalInput").ap()
    whh = nc.dram_tensor("whh", [H, 3 * HS], f32r, kind="ExternalInput").ap()
    bi = nc.dram_tensor("bi", [1, 3 * HS], f32r, kind="ExternalInput").ap()
    bh = nc.dram_tensor("bh", [1, 3 * HS], f32r, kind="ExternalInput").ap()
    bo = nc.dram_tensor("bo", [1, H], bf16, kind="ExternalInput").ap()
    ones = nc.dram_tensor("ones", [1, N], f32r, kind="ExternalInput").ap()
    onesbf = nc.dram_tensor("onesbf", [1, NB], bf16, kind="ExternalInput").ap()
    eye = nc.dram_tensor("eye", [128, 128], f32, kind="ExternalInput").ap()
    wattn = nc.dram_tensor("wattn", [H, H], f32r, kind="ExternalInput").ap()
    woutT = nc.dram_tensor("woutT", [2 * H, H], bf16, kind="ExternalInput").ap()
    ec = nc.dram_tensor("ec", [NB, S, H], bf16, kind="ExternalInput").ap()
    et = nc.dram_tensor("et", [NB, H, S], f32r, kind="ExternalInput").ap()

    hnew_o = nc.dram_tensor("hnew_o", [N, HS], f32, kind="ExternalOutput").ap()
    ctx_o = nc.dram_tensor("ctx_o", [NB, H], f32, kind="ExternalOutput").ap()
    attn_o = nc.dram_tensor("attn_o", [NB, S], f32, kind="ExternalOutput").ap()
    out_o = nc.dram_tensor("out_o", [NB, H], f32, kind="ExternalOutput").ap()

    with tile.TileContext(nc) as tc, ExitStack() as ctx:
        sbR = ctx.enter_context(tc.tile_pool(name="sbR", bufs=1))
        sbW = ctx.enter_context(tc.tile_pool(name="sbW", bufs=3))
        sbS = ctx.enter_context(tc.tile_pool(name="sbS", bufs=1))
        sbE = ctx.enter_context(tc.tile_pool(name="sbE", bufs=NB))
        psB = ctx.enter_context(tc.tile_pool(name="psB", bufs=2, space="PSUM"))
        psS = ctx.enter_context(tc.tile_pool(name="psS", bufs=2, space="PSUM"))
        psT = ctx.enter_context(tc.tile_pool(name="psT", bufs=2, space="PSUM"))
        dram = ctx.enter_context(tc.tile_pool(name="dram", bufs=2, space="DRAM"))

        # ---- resident small loads ----
        xT_sb = sbR.tile([128, KX, N], f32r)
        nc.sync.dma_start(xT_sb[:], xT.rearrange("(k p) m -> p k m", p=128))
        hT_sb = sbR.tile([128, KH, N], f32r)
        nc.sync.dma_start(hT_sb[:], hT.rearrange("(k p) m -> p k m", p=128))
        hsl_sb = sbR.tile([N, HS], f32)
        nc.sync.dma_start(hsl_sb[:], hsl)
        eye_sb = sbR.tile([128, 128], f32)
        nc.sync.dma_start(eye_sb[:], eye)
        bi_sb = sbR.tile([1, 3 * HS], f32r)
        nc.sync.dma_start(bi_sb[:], bi)
        bh_sb = sbR.tile([1, 3 * HS], f32r)
        nc.sync.dma_start(bh_sb[:], bh)
        bo_sb = sbR.tile([1, H], bf16)
        nc.sync.dma_start(bo_sb[:], bo)
        ones_sb = sbR.tile([1, N], f32r)
        nc.sync.dma_start(ones_sb[:], ones)
        onesbf_sb = sbR.tile([1, NB], bf16)
        nc.sync.dma_start(onesbf_sb[:], onesbf)

        # ---- encoder shard loads (resident, overlap with everything) ----
        et_tiles = []
        ec_tiles = []
        for n in range(NB):
            t = sbE.tile([128, KH, S], f32r, tag="et")
            nc.sync.dma_start(t[:], et[n].rearrange("(k p) s -> p k s", p=128))
            et_tiles.append(t)
            t2 = sbE.tile([128, 2, H], bf16, tag="ec")
            nc.sync.dma_start(t2[:], ec[n].rearrange("(sc p) h -> p sc h", p=128))
            ec_tiles.append(t2)

        # ---- GRU: gi/gh for gate-column slice, all 64 batches ----
        wih_r = wih.rearrange("(k p) g -> k p g", p=128)
        whh_r = whh.rearrange("(k p) g -> k p g", p=128)
        gi = psS.tile([N, 3 * HS], f32, tag="g")
        for k in range(KX):
            w_t = sbW.tile([128, 3 * HS], f32r, tag="wih")
            nc.sync.dma_start(w_t[:], wih_r[k])
            nc.tensor.matmul(gi[:], xT_sb[:, k, :], w_t[:], start=(k == 0), stop=False)
        nc.tensor.matmul(gi[:], ones_sb[:], bi_sb[:], start=False, stop=True)
        gh = psS.tile([N, 3 * HS], f32, tag="g")
        for k in range(KH):
            w_t = sbW.tile([128, 3 * HS], f32r, tag="whh")
            nc.sync.dma_start(w_t[:], whh_r[k])
            nc.tensor.matmul(gh[:], hT_sb[:, k, :], w_t[:], start=(k == 0), stop=False)
        nc.tensor.matmul(gh[:], ones_sb[:], bh_sb[:], start=False, stop=True)

        AF = mybir.ActivationFunctionType
        # DVE tensor_tensor can take at most one PSUM operand; stage gh in SBUF
        gh_sb = sbS.tile([N, 3 * HS], f32, tag="ghsb")
        nc.scalar.copy(gh_sb[:], gh[:])
        t1 = sbS.tile([N, HS], f32, tag="t1")
        nc.vector.tensor_add(t1[:], gi[:, 0:HS], gh_sb[:, 0:HS])
        r = sbS.tile([N, HS], f32, tag="r")
        nc.scalar.activation(r[:], t1[:], AF.Sigmoid)
        t2 = sbS.tile([N, HS], f32, tag="t2")
        nc.vector.tensor_add(t2[:], gi[:, HS:2 * HS], gh_sb[:, HS:2 * HS])
        z = sbS.tile([N, HS], f32, tag="z")
        nc.scalar.activation(z[:], t2[:], AF.Sigmoid)
        t3 = sbS.tile([N, HS], f32, tag="t3")
        nc.vector.tensor_mul(t3[:], r[:], gh_sb[:, 2 * HS:3 * HS])
        t4 = sbS.tile([N, HS], f32, tag="t4")
        nc.vector.tensor_add(t4[:], t3[:], gi[:, 2 * HS:3 * HS])
        nn_t = sbS.tile([N, HS], f32, tag="nn")
        nc.scalar.activation(nn_t[:], t4[:], AF.Tanh)
        d1 = sbS.tile([N, HS], f32, tag="d1")
        nc.vector.tensor_sub(d1[:], hsl_sb[:], nn_t[:])
        d2 = sbS.tile([N, HS], f32, tag="d2")
        nc.vector.tensor_mul(d2[:], z[:], d1[:])
        h_new = sbS.tile([N, HS], f32, tag="hn")
        nc.vector.tensor_add(h_new[:], nn_t[:], d2[:])
        nc.sync.dma_start(hnew_o, h_new[:])

        # ---- transpose h_new and AllToAll -> rnnT_own [128, KH, NB] ----
        trp = psT.tile([128, N], f32, tag="tr")
        nc.tensor.transpose(trp[:], h_new[:], eye_sb[0:N, 0:N])
        hnT = sbS.tile([128, N], f32r, tag="hnT")
        nc.scalar.copy(hnT[:], trp[:])
        g_in = dram.tile([NCORES, 128, NB], f32r)
        for j in range(NCORES):
            nc.sync.dma_start(g_in[j], hnT[:, j * NB:(j + 1) * NB])
        g_out = dram.tile([NCORES, 128, NB], f32r)
        nc.gpsimd.collective_compute(
            "AllToAll", mybir.AluOpType.bypass,
            replica_groups=[list(range(NCORES))],
            ins=[g_in.opt()], outs=[g_out.opt()],
        )
        rnnT = sbS.tile([128, KH, NB], f32r, tag="rnnT")
        for j in range(NCORES):
            nc.sync.dma_start(rnnT[:, j, :], g_out[j])

        # ---- q = rnn_own @ W_attn  [NB, H] ----
        wattn_r = wattn.rearrange("(k p) h -> k p h", p=128)
        q_ps = psB.tile([NB, H], f32, tag="big")
        for k in range(KH):
            w_t = sbW.tile([128, H], f32r, tag="wattn")
            nc.sync.dma_start(w_t[:], wattn_r[k])
            for h2 in range(2):
                nc.tensor.matmul(q_ps[:, h2 * 512:(h2 + 1) * 512],
                                 rnnT[:, k, :], w_t[:, h2 * 512:(h2 + 1) * 512],
                                 start=(k == 0), stop=(k == KH - 1))
        q_sb = sbS.tile([NB, H], f32, tag="qsb")
        nc.scalar.copy(q_sb[:], q_ps[:])
        qT = sbS.tile([128, KH, NB], f32r, tag="qT")
        for k in range(KH):
            tq = psT.tile([128, NB], f32, tag="tr")
            nc.tensor.transpose(tq[:], q_sb[:, k * 128:(k + 1) * 128], eye_sb[0:NB, 0:NB])
            nc.scalar.copy(qT[:, k, :], tq[:])

        # ---- scores: per batch matvec over E^T chunks; matmul output must sit
        # at PSUM base partition 0, so each batch accumulates in a [1, S] tile
        # and a SBUF->SBUF DMA moves it onto row n of the packed tile ----
        scores_sb = sbS.tile([NB, S], f32, tag="scb")
        for n in range(NB):
            sc_n = psT.tile([1, S], f32, tag="tr")
            for k in range(KH):
                nc.tensor.matmul(sc_n[:], qT[:, k, n:n + 1],
                                 et_tiles[n][:, k, :],
                                 start=(k == 0), stop=(k == KH - 1))
            st = sbW.tile([1, S], f32, tag="scst")
            nc.scalar.copy(st[:], sc_n[:])
            nc.sync.dma_start(scores_sb[n:n + 1, :], st[:])

        # ---- softmax over S (vectorized across the 8 batch rows) ----
        mx = sbS.tile([NB, 1], f32, tag="mx")
        nc.vector.tensor_reduce(mx[:], scores_sb[:], axis=mybir.AxisListType.X,
                                op=mybir.AluOpType.max)
        sub = sbS.tile([NB, S], f32, tag="sub")
        nc.vector.tensor_scalar_sub(sub[:], scores_sb[:], mx[:])
        ex = sbS.tile([NB, S], f32, tag="ex")
        nc.scalar.activation(ex[:], sub[:], AF.Exp)
        sm = sbS.tile([NB, 1], f32, tag="sm")
        nc.vector.tensor_reduce(sm[:], ex[:], axis=mybir.AxisListType.X,
                                op=mybir.AluOpType.add)
        rc = sbS.tile([NB, 1], f32, tag="rc")
        nc.vector.reciprocal(rc[:], sm[:])
        attn = sbS.tile([NB, S], f32, tag="attn")
        nc.vector.tensor_scalar_mul(attn[:], ex[:], rc[:])
        nc.sync.dma_start(attn_o, attn[:])
        attnT = sbS.tile([128, 2, NB], bf16, tag="attnT")
        for s2 in range(2):
            ta = psT.tile([128, NB], f32, tag="tr")
            nc.tensor.transpose(ta[:], attn[:, s2 * 128:(s2 + 1) * 128],
                                eye_sb[0:NB, 0:NB])
            nc.scalar.copy(attnT[:, s2, :], ta[:])

        # ---- context = attn @ E  [NB, H]; same base-partition-0 dance ----
        ctx_sb = sbS.tile([NB, H], f32, tag="ctx")
        for n in range(NB):
            cx_n = psB.tile([1, H], f32, tag="big")
            for s2 in range(2):
                for h2 in range(2):
                    nc.tensor.matmul(cx_n[:, h2 * 512:(h2 + 1) * 512],
                                     attnT[:, s2, n:n + 1],
                                     ec_tiles[n][:, s2, h2 * 512:(h2 + 1) * 512],
                                     start=(s2 == 0), stop=(s2 == 1))
            cst = sbW.tile([1, H], f32, tag="cxst")
            nc.scalar.copy(cst[:], cx_n[:])
            nc.sync.dma_start(ctx_sb[n:n + 1, :], cst[:])
        nc.sync.dma_start(ctx_o, ctx_sb[:])

        # ---- output = cat(rnn, context) @ W_out.T + b_out  [NB, H] ----
        rnbf = sbS.tile([128, KH, NB], bf16, tag="rnbf")
        nc.vector.tensor_copy(rnbf[:], rnnT[:])
        ctxT = sbS.tile([128, KH, NB], bf16, tag="ctxT")
        for k in range(KH):
            tcx = psT.tile([128, NB], f32, tag="tr")
            nc.tensor.transpose(tcx[:], ctx_sb[:, k * 128:(k + 1) * 128],
                                eye_sb[0:NB, 0:NB])
            nc.scalar.copy(ctxT[:, k, :], tcx[:])
        woutT_r = woutT.rearrange("(k p) h -> k p h", p=128)
        out_ps = psB.tile([NB, H], f32, tag="big")
        for k in range(2 * KH):
            w_t = sbW.tile([128, H], bf16, tag="wout")
            nc.sync.dma_start(w_t[:], woutT_r[k])
            lhsT = rnbf[:, k, :] if k < KH else ctxT[:, k - KH, :]
            for h2 in range(2):
                nc.tensor.matmul(out_ps[:, h2 * 512:(h2 + 1) * 512],
                                 lhsT, w_t[:, h2 * 512:(h2 + 1) * 512],
                                 start=(k == 0), stop=False)
        for h2 in range(2):
            nc.tensor.matmul(out_ps[:, h2 * 512:(h2 + 1) * 512],
                             onesbf_sb[:], bo_sb[:, h2 * 512:(h2 + 1) * 512],
                             start=False, stop=True)
        out_sb = sbS.tile([NB, H], f32, tag="out")
        nc.scalar.copy(out_sb[:], out_ps[:])
        nc.sync.dma_start(out_o, out_sb[:])

    nc.compile()
    return nc


def kernel(last_output, last_context, last_hidden, encoder_outputs,
           W_ih, b_ih, W_hh, b_hh, W_attn, b_attn, W_out, b_out):
    global _last_results
    from concourse.bass_utils import run_bass_kernel_spmd

    if "nc" not in _cache:
        _cache["nc"] = _build()
    nc = _cache["nc"]

    f32 = np.float32
    h0 = np.asarray(last_hidden, f32)[0]                      # [N, H]
    x = np.concatenate([np.asarray(last_output, f32),
                        np.asarray(last_context, f32)], axis=1)
    xT = np.ascontiguousarray(x.T)                            # [2H, N]
    hT = np.ascontiguousarray(h0.T)                           # [H, N]
    enc = np.asarray(encoder_outputs, f32)                    # [N, S, H]
    W_ih = np.asarray(W_ih, f32); W_hh = np.asarray(W_hh, f32)
    W_attn = np.asarray(W_attn, f32); W_out = np.asarray(W_out, f32)
    woutT = np.ascontiguousarray(W_out.T).astype(BF16)        # [2H, H]
    eye = np.eye(128, dtype=f32)
    ones = np.ones((1, N), f32)
    onesbf = np.ones((1, NB), BF16)
    bo = np.asarray(b_out, f32).reshape(1, H).astype(BF16)

    in_maps = []
    for c in range(NCORES):
        cols = np.concatenate([g * H + np.arange(c * HS, (c + 1) * HS)
                               for g in range(3)])
        enc_c = enc[c * NB:(c + 1) * NB]
        in_maps.append({
            "xT": xT, "hT": hT,
            "hsl": np.ascontiguousarray(h0[:, c * HS:(c + 1) * HS]),
            "wih": np.ascontiguousarray(W_ih[cols, :].T),
            "whh": np.ascontiguousarray(W_hh[cols, :].T),
            "bi": np.ascontiguousarray(b_ih[cols]).reshape(1, -1).astype(f32),
            "bh": np.ascontiguousarray(b_hh[cols]).reshape(1, -1).astype(f32),
            "bo": bo, "ones": ones, "onesbf": onesbf, "eye": eye,
            "wattn": W_attn, "woutT": woutT,
            "ec": enc_c.astype(BF16),
            "et": np.ascontiguousarray(enc_c.transpose(0, 2, 1)),
        })

    res = run_bass_kernel_spmd(nc, in_maps, core_ids=list(range(NCORES)))
    _last_results = res
    outs = res.results
    output = np.concatenate([outs[c]["out_o"] for c in range(NCORES)], axis=0)
    context = np.concatenate([outs[c]["ctx_o"] for c in range(NCORES)], axis=0)
    hidden = np.concatenate([outs[c]["hnew_o"] for c in range(NCORES)], axis=1)[None]
    attn_w = np.concatenate([outs[c]["attn_o"] for c in range(NCORES)],
                            axis=0).reshape(N, 1, S)
    return (output.astype(f32), context.astype(f32),
            hidden.astype(f32), attn_w.astype(f32))


# revision 12
# speedup vs baseline: 1.0195x; 1.0195x over previous
"""Trainium2 Bass kernel for a single-step attention decoder RNN (AttnDecoderRNN).

Math (reference semantics, N=64 batch, S=256 src len, H=1024 hidden):
  GRU step (PyTorch gate order r,z,n) -> h_new
  energy = enc @ W_attn.T + b_attn ; scores = einsum('nh,nsh->ns', h_new, energy)
  attn = softmax(scores) ; context = einsum('ns,nsh->nh', attn, enc)
  output = cat(h_new, context) @ W_out.T + b_out

Algebraic restructure: scores[n,s] = enc[n,s,:] . q[n,:] + h_new[n,:].b_attn with
q = h_new @ W_attn. The per-row constant h_new.b_attn cancels in softmax, so the
[N,S,H] energy tensor is never materialized (34 GFLOP -> 1.7 GFLOP).

Distribution over 8 NeuronCores:
  - GRU tensor-parallel over the hidden dim: core c computes gate columns
    [128c:128c+128) of r,z,n for ALL 64 batches (weights pre-sharded on host).
  - An AllToAll exchanges [128-hidden-slice x 8-batch-block] tiles so each core
    ends up with rnn^T [1024, 8] for ITS 8 batches (no dynamic indexing).
  - Attention + output GEMM are batch-parallel (8 batches/core).
Matmuls run as float32r (reduced-precision fp32, 1 cyc/row, ~1.6e-4 rel err);
the context/output GEMMs use bf16 operands (their outputs tolerate ~2e-3).
"""
import numpy as np
import ml_dtypes
from contextlib import ExitStack

N, S, H = 64, 256, 1024
NCORES = 8
NB = N // NCORES        # 8 batches per core
HS = H // NCORES        # 128 hidden columns per core
KX = 2 * H // 128       # 16 contraction chunks for W_ih
KH = H // 128           # 8 contraction chunks for W_hh / W_attn
BF16 = ml_dtypes.bfloat16

_cache = {}
_last_results = None


def _build():
    import concourse.tile as tile
    from concourse import bacc, mybir

    f32 = mybir.dt.float32
    f32r = mybir.dt.float32r
    bf16 = mybir.dt.bfloat16

    nc = bacc.Bacc("TRN2", target_bir_lowering=False, debug=False,
                   enable_asserts=False, num_devices=NCORES)

    xT = nc.dram_tensor("xT", [2 * H, N], f32r, kind="ExternalInput").ap()
    hT = nc.dram_tensor("hT", [H, N], f32r, kind="ExternalInput").ap()
    hsl = nc.dram_tensor("hsl", [N, HS], f32, kind="ExternalInput").ap()
    wih = nc.dram_tensor("wih", [2 * H, 3 * HS], f32r, kind="ExternalInput").ap()
    whh = nc.dram_tensor("whh", [H, 3 * HS], f32r, kind="ExternalInput").ap()
    bi = nc.dram_tensor("bi", [1, 3 * HS], f32r, kind="ExternalInput").ap()
    bh = nc.dram_tensor("bh", [1, 3 * HS], f32r, kind="ExternalInput").ap()
    bo = nc.dram_tensor("bo", [1, H], bf16, kind="ExternalInput").ap()
    ones = nc.dram_tensor("ones", [1, N], f32r, kind="ExternalInput").ap()
    onesbf = nc.dram_tensor("onesbf", [1, NB], bf16, kind="ExternalInput").ap()
    eye = nc.dram_tensor("eye", [128, 128], f32, kind="ExternalInput").ap()
    wattn = nc.dram_tensor("wattn", [128, H], f32r, kind="ExternalInput").ap()
    woutT = nc.dram_tensor("woutT", [2 * H, H], bf16, kind="ExternalInput").ap()
    ec = nc.dram_tensor("ec", [NB, S, H], bf16, kind="ExternalInput").ap()
    et = nc.dram_tensor("et", [NB, H, S], f32r, kind="ExternalInput").ap()

    hnew_o = nc.dram_tensor("hnew_o", [N, HS], f32, kind="ExternalOutput").ap()
    ctx_o = nc.dram_tensor("ctx_o", [NB, H], f32, kind="ExternalOutput").ap()
    attn_o = nc.dram_tensor("attn_o", [NB, S], f32, kind="ExternalOutput").ap()
    out_o = nc.dram_tensor("out_o", [NB, H], f32, kind="ExternalOutput").ap()

    with tile.TileContext(nc) as tc, ExitStack() as ctx:
        sbR = ctx.enter_context(tc.tile_pool(name="sbR", bufs=1))
        sbW = ctx.enter_context(tc.tile_pool(name="sbW", bufs=8))
        sbO = ctx.enter_context(tc.tile_pool(name="sbO", bufs=4))
        sbX = ctx.enter_context(tc.tile_pool(name="sbX", bufs=2))
        sbS = ctx.enter_context(tc.tile_pool(name="sbS", bufs=1))
        sbE = ctx.enter_context(tc.tile_pool(name="sbE", bufs=NB))
        psB = ctx.enter_context(tc.tile_pool(name="psB", bufs=2, space="PSUM"))
        psS = ctx.enter_context(tc.tile_pool(name="psS", bufs=2, space="PSUM"))
        psT = ctx.enter_context(tc.tile_pool(name="psT", bufs=2, space="PSUM"))
        dram = ctx.enter_context(tc.tile_pool(name="dram", bufs=2, space="DRAM"))

        # ---- resident small loads ----
        xT_sb = sbR.tile([128, KX, N], f32r)
        nc.sync.dma_start(xT_sb[:], xT.rearrange("(k p) m -> p k m", p=128))
        hT_sb = sbR.tile([128, KH, N], f32r)
        nc.sync.dma_start(hT_sb[:], hT.rearrange("(k p) m -> p k m", p=128))
        hsl_sb = sbR.tile([N, HS], f32)
        nc.sync.dma_start(hsl_sb[:], hsl)
        eye_sb = sbR.tile([128, 128], f32)
        nc.sync.dma_start(eye_sb[:], eye)
        bi_sb = sbR.tile([1, 3 * HS], f32r)
        nc.sync.dma_start(bi_sb[:], bi)
        bh_sb = sbR.tile([1, 3 * HS], f32r)
        nc.sync.dma_start(bh_sb[:], bh)
        bo_sb = sbR.tile([1, H], bf16)
        nc.sync.dma_start(bo_sb[:], bo)
        ones_sb = sbR.tile([1, N], f32r)
        nc.sync.dma_start(ones_sb[:], ones)
        onesbf_sb = sbR.tile([1, NB], bf16)
        nc.sync.dma_start(onesbf_sb[:], onesbf)

        # ---- encoder shard loads (issued after the collectives so the GRU
        # weight stream owns the DMA ring early; still fully overlapped) ----
        et_tiles = []
        ec_tiles = []
        for n in range(NB):
            t = sbE.tile([128, KH, S], f32r, tag="et")
            nc.sync.dma_start(t[:], et[n].rearrange("(k p) s -> p k s", p=128))
            et_tiles.append(t)
            t2 = sbE.tile([128, 2, H], bf16, tag="ec")
            nc.sync.dma_start(t2[:], ec[n].rearrange("(sc p) h -> p sc h", p=128))
            ec_tiles.append(t2)

        # ---- GRU: gi/gh for gate-column slice, all 64 batches ----
        wih_r = wih.rearrange("(k p) g -> k p g", p=128)
        whh_r = whh.rearrange("(k p) g -> k p g", p=128)
        gi = psS.tile([N, 3 * HS], f32, tag="g")
        for k in range(KX):
            w_t = sbW.tile([128, 3 * HS], f32r, tag="wih")
            nc.sync.dma_start(w_t[:], wih_r[k])
            nc.tensor.matmul(gi[:], xT_sb[:, k, :], w_t[:], start=(k == 0), stop=False)
        nc.tensor.matmul(gi[:], ones_sb[:], bi_sb[:], start=False, stop=True)
        gh = psS.tile([N, 3 * HS], f32, tag="g")
        for k in range(KH):
            w_t = sbW.tile([128, 3 * HS], f32r, tag="whh")
            nc.sync.dma_start(w_t[:], whh_r[k])
            nc.tensor.matmul(gh[:], hT_sb[:, k, :], w_t[:], start=(k == 0), stop=False)
        nc.tensor.matmul(gh[:], ones_sb[:], bh_sb[:], start=False, stop=True)

        AF = mybir.ActivationFunctionType
        # DVE tensor_tensor can take at most one PSUM operand; stage gh in SBUF
        gh_sb = sbS.tile([N, 3 * HS], f32, tag="ghsb")
        nc.scalar.copy(gh_sb[:], gh[:])
        t1 = sbS.tile([N, HS], f32, tag="t1")
        nc.vector.tensor_add(t1[:], gi[:, 0:HS], gh_sb[:, 0:HS])
        r = sbS.tile([N, HS], f32, tag="r")
        nc.scalar.activation(r[:], t1[:], AF.Sigmoid)
        t2 = sbS.tile([N, HS], f32, tag="t2")
        nc.vector.tensor_add(t2[:], gi[:, HS:2 * HS], gh_sb[:, HS:2 * HS])
        z = sbS.tile([N, HS], f32, tag="z")
        nc.scalar.activation(z[:], t2[:], AF.Sigmoid)
        t3 = sbS.tile([N, HS], f32, tag="t3")
        nc.vector.tensor_mul(t3[:], r[:], gh_sb[:, 2 * HS:3 * HS])
        t4 = sbS.tile([N, HS], f32, tag="t4")
        nc.vector.tensor_add(t4[:], t3[:], gi[:, 2 * HS:3 * HS])
        nn_t = sbS.tile([N, HS], f32, tag="nn")
        nc.scalar.activation(nn_t[:], t4[:], AF.Tanh)
        d1 = sbS.tile([N, HS], f32, tag="d1")
        nc.vector.tensor_sub(d1[:], hsl_sb[:], nn_t[:])
        d2 = sbS.tile([N, HS], f32, tag="d2")
        nc.vector.tensor_mul(d2[:], z[:], d1[:])
        h_new = sbS.tile([N, HS], f32, tag="hn")
        nc.vector.tensor_add(h_new[:], nn_t[:], d2[:])
        nc.sync.dma_start(hnew_o, h_new[:])

        # ---- transpose h_new and AllToAll -> rnnT_own [128, KH, NB] ----
        trp = psT.tile([128, N], f32, tag="tr")
        nc.tensor.transpose(trp[:], h_new[:], eye_sb[0:N, 0:N])
        hnT = sbS.tile([128, N], f32r, tag="hnT")
        nc.scalar.copy(hnT[:], trp[:])
        # ---- q tensor-parallel: partial q for ALL 64 batches from this
        # core's hidden slice (lhsT = h_new^T already in SBUF) ----
        wattn_sb = sbR.tile([128, H], f32r)
        nc.sync.dma_start(wattn_sb[:], wattn)
        qp_ps = psB.tile([N, H], f32, tag="big")
        for h2 in range(2):
            nc.tensor.matmul(qp_ps[:, h2 * 512:(h2 + 1) * 512],
                             hnT[:], wattn_sb[:, h2 * 512:(h2 + 1) * 512],
                             start=True, stop=True)
        qp_sb = sbS.tile([N, H], f32, tag="qpsb")
        nc.scalar.copy(qp_sb[:], qp_ps[:])

        # ---- ONE AllToAll carries both payloads (two concurrent collectives
        # cross-contaminate on HW): block j = [h_newT cols 8j:8j+8 (1024 el),
        # partial-q rows 8j:8j+8 (8192 el)]. After A2A, block j holds rank
        # j's hidden chunk + rank j's partial q for OUR batches. ----
        PL = 128 * NB + NB * H
        g_in = dram.tile([NCORES, PL], f32)
        for j in range(NCORES):
            nc.gpsimd.dma_start(
                g_in[j, 0:128 * NB].rearrange("(p e) -> p e", p=128),
                hnT[:, j * NB:(j + 1) * NB])
            nc.sync.dma_start(
                g_in[j, 128 * NB:PL].rearrange("(p e) -> p e", p=NB),
                qp_sb[j * NB:(j + 1) * NB, :])
        g_out = dram.tile([NCORES, PL], f32)
        nc.gpsimd.collective_compute(
            "AllToAll", mybir.AluOpType.bypass,
            replica_groups=[list(range(NCORES))],
            ins=[g_in.opt()], outs=[g_out.opt()],
        )
        rnnT = sbS.tile([128, KH, NB], f32r, tag="rnnT")
        for j in range(NCORES):
            nc.gpsimd.dma_start(rnnT[:, j, :],
                              g_out[j, 0:128 * NB].rearrange("(p e) -> p e", p=128))
        # sum the 8 partial-q blocks -> q_own [NB, H]
        qtmp = []
        for j in range(NCORES):
            t = sbX.tile([NB, H], f32, tag="qtmp")
            nc.sync.dma_start(t[:], g_out[j, 128 * NB:PL].rearrange("(p e) -> p e", p=NB))
            qtmp.append(t)
        qs1 = []
        for j in range(4):
            a = sbX.tile([NB, H], f32, tag="qs1")
            nc.vector.tensor_add(a[:], qtmp[2 * j][:], qtmp[2 * j + 1][:])
            qs1.append(a)
        qs2 = []
        for j in range(2):
            a = sbX.tile([NB, H], f32, tag="qs2")
            nc.vector.tensor_add(a[:], qs1[2 * j][:], qs1[2 * j + 1][:])
            qs2.append(a)
        q_sb = sbS.tile([NB, H], f32, tag="qsb")
        nc.vector.tensor_add(q_sb[:], qs2[0][:], qs2[1][:])
        qT = sbS.tile([128, KH, NB], f32r, tag="qT")
        for k in range(KH):
            tq = psT.tile([128, NB], f32, tag="tr")
            nc.tensor.transpose(tq[:], q_sb[:, k * 128:(k + 1) * 128], eye_sb[0:NB, 0:NB])
            nc.scalar.copy(qT[:, k, :], tq[:])

        # ---- scores: per batch matvec over E^T chunks; matmul output must sit
        # at PSUM base partition 0, so each batch accumulates in a [1, S] tile
        # and a SBUF->SBUF DMA moves it onto row n of the packed tile ----
        scores_sb = sbS.tile([NB, S], f32, tag="scb")
        for n in range(NB):
            sc_n = psT.tile([1, S], f32, tag="tr")
            for k in range(KH):
                nc.tensor.matmul(sc_n[:], qT[:, k, n:n + 1],
                                 et_tiles[n][:, k, :],
                                 start=(k == 0), stop=(k == KH - 1))
            st = sbX.tile([1, S], f32, tag="scst")
            nc.scalar.copy(st[:], sc_n[:])
            nc.sync.dma_start(scores_sb[n:n + 1, :], st[:])

        # ---- softmax over S (vectorized across the 8 batch rows) ----
        mx = sbS.tile([NB, 1], f32, tag="mx")
        nc.vector.tensor_reduce(mx[:], scores_sb[:], axis=mybir.AxisListType.X,
                                op=mybir.AluOpType.max)
        sub = sbS.tile([NB, S], f32, tag="sub")
        nc.vector.tensor_scalar_sub(sub[:], scores_sb[:], mx[:])
        ex = sbS.tile([NB, S], f32, tag="ex")
        nc.scalar.activation(ex[:], sub[:], AF.Exp)
        sm = sbS.tile([NB, 1], f32, tag="sm")
        nc.vector.tensor_reduce(sm[:], ex[:], axis=mybir.AxisListType.X,
                                op=mybir.AluOpType.add)
        rc = sbS.tile([NB, 1], f32, tag="rc")
        nc.vector.reciprocal(rc[:], sm[:])
        attn = sbS.tile([NB, S], f32, tag="attn")
        nc.vector.tensor_scalar_mul(attn[:], ex[:], rc[:])
        nc.sync.dma_start(attn_o, attn[:])
        attnT = sbS.tile([128, 2, NB], bf16, tag="attnT")
        for s2 in range(2):
            ta = psT.tile([128, NB], f32, tag="tr")
            nc.tensor.transpose(ta[:], attn[:, s2 * 128:(s2 + 1) * 128],
                                eye_sb[0:NB, 0:NB])
            nc.scalar.copy(attnT[:, s2, :], ta[:])

        # ---- context = attn @ E  [NB, H]; same base-partition-0 dance ----
        ctx_sb = sbS.tile([NB, H], f32, tag="ctx")
        for n in range(NB):
            cx_n = psB.tile([1, H], f32, tag="big")
            for s2 in range(2):
                for h2 in range(2):
                    nc.tensor.matmul(cx_n[:, h2 * 512:(h2 + 1) * 512],
                                     attnT[:, s2, n:n + 1],
                                     ec_tiles[n][:, s2, h2 * 512:(h2 + 1) * 512],
                                     start=(s2 == 0), stop=(s2 == 1))
            cst = sbX.tile([1, H], f32, tag="cxst")
            nc.scalar.copy(cst[:], cx_n[:])
            nc.sync.dma_start(ctx_sb[n:n + 1, :], cst[:])
        nc.sync.dma_start(ctx_o, ctx_sb[:])

        # ---- output = cat(rnn, context) @ W_out.T + b_out  [NB, H] ----
        rnbf = sbS.tile([128, KH, NB], bf16, tag="rnbf")
        nc.vector.tensor_copy(rnbf[:], rnnT[:])
        ctxT = sbS.tile([128, KH, NB], bf16, tag="ctxT")
        for k in range(KH):
            tcx = psT.tile([128, NB], f32, tag="tr")
            nc.tensor.transpose(tcx[:], ctx_sb[:, k * 128:(k + 1) * 128],
                                eye_sb[0:NB, 0:NB])
            nc.scalar.copy(ctxT[:, k, :], tcx[:])
        woutT_r = woutT.rearrange("(k p) h -> k p h", p=128)
        out_ps = psB.tile([NB, H], f32, tag="big")
        for k in range(2 * KH):
            w_t = sbO.tile([128, H], bf16, tag="wout")
            nc.sync.dma_start(w_t[:], woutT_r[k])
            lhsT = rnbf[:, k, :] if k < KH else ctxT[:, k - KH, :]
            for h2 in range(2):
                nc.tensor.matmul(out_ps[:, h2 * 512:(h2 + 1) * 512],
                                 lhsT, w_t[:, h2 * 512:(h2 + 1) * 512],
                                 start=(k == 0), stop=False)
        for h2 in range(2):
            nc.tensor.matmul(out_ps[:, h2 * 512:(h2 + 1) * 512],
                             onesbf_sb[:], bo_sb[:, h2 * 512:(h2 + 1) * 512],
                             start=False, stop=True)
        out_sb = sbS.tile([NB, H], f32, tag="out")
        nc.scalar.copy(out_sb[:], out_ps[:])
        nc.sync.dma_start(out_o, out_sb[:])

    nc.compile()
    return nc


def kernel(last_output, last_context, last_hidden, encoder_outputs,
           W_ih, b_ih, W_hh, b_hh, W_attn, b_attn, W_out, b_out):
    global _last_results
    from concourse.bass_utils import run_bass_kernel_spmd

    if "nc" not in _cache:
        _cache["nc"] = _build()
    nc = _cache["nc"]

    f32 = np.float32
    h0 = np.asarray(last_hidden, f32)[0]                      # [N, H]
    x = np.concatenate([np.asarray(last_output, f32),
                        np.asarray(last_context, f32)], axis=1)
    xT = np.ascontiguousarray(x.T)                            # [2H, N]
    hT = np.ascontiguousarray(h0.T)                           # [H, N]
    enc = np.asarray(encoder_outputs, f32)                    # [N, S, H]
    W_ih = np.asarray(W_ih, f32); W_hh = np.asarray(W_hh, f32)
    W_attn = np.asarray(W_attn, f32); W_out = np.asarray(W_out, f32)
    woutT = np.ascontiguousarray(W_out.T).astype(BF16)        # [2H, H]
    eye = np.eye(128, dtype=f32)
    ones = np.ones((1, N), f32)
    onesbf = np.ones((1, NB), BF16)
    bo = np.asarray(b_out, f32).reshape(1, H).astype(BF16)

    in_maps = []
    for c in range(NCORES):
        cols = np.concatenate([g * H + np.arange(c * HS, (c + 1) * HS)
                               for g in range(3)])
        enc_c = enc[c * NB:(c + 1) * NB]
        in_maps.append({
            "xT": xT, "hT": hT,
            "hsl": np.ascontiguousarray(h0[:, c * HS:(c + 1) * HS]),
            "wih": np.ascontiguousarray(W_ih[cols, :].T),
            "whh": np.ascontiguousarray(W_hh[cols, :].T),
            "bi": np.ascontiguousarray(b_ih[cols]).reshape(1, -1).astype(f32),
            "bh": np.ascontiguousarray(b_hh[cols]).reshape(1, -1).astype(f32),
            "bo": bo, "ones": ones, "onesbf": onesbf, "eye": eye,
            "wattn": np.ascontiguousarray(W_attn[c * HS:(c + 1) * HS, :]), "woutT": woutT,
            "ec": enc_c.astype(BF16),
            "et": np.ascontiguousarray(enc_c.transpose(0, 2, 1)),
        })

    res = run_bass_kernel_spmd(nc, in_maps, core_ids=list(range(NCORES)))
    _last_results = res
    outs = res.results
    output = np.concatenate([outs[c]["out_o"] for c in range(NCORES)], axis=0)
    context = np.concatenate([outs[c]["ctx_o"] for c in range(NCORES)], axis=0)
    hidden = np.concatenate([outs[c]["hnew_o"] for c in range(NCORES)], axis=1)[None]
    attn_w = np.concatenate([outs[c]["attn_o"] for c in range(NCORES)],
                            axis=0).reshape(N, 1, S)
    return (output.astype(f32), context.astype(f32),
            hidden.astype(f32), attn_w.astype(f32))


# revision 16
# speedup vs baseline: 1.0258x; 1.0061x over previous
"""Trainium2 Bass kernel for a single-step attention decoder RNN (AttnDecoderRNN).

Math (reference semantics, N=64 batch, S=256 src len, H=1024 hidden):
  GRU step (PyTorch gate order r,z,n) -> h_new
  energy = enc @ W_attn.T + b_attn ; scores = einsum('nh,nsh->ns', h_new, energy)
  attn = softmax(scores) ; context = einsum('ns,nsh->nh', attn, enc)
  output = cat(h_new, context) @ W_out.T + b_out

Algebraic restructure: scores[n,s] = enc[n,s,:] . q[n,:] + h_new[n,:].b_attn with
q = h_new @ W_attn. The per-row constant h_new.b_attn cancels in softmax, so the
[N,S,H] energy tensor is never materialized (34 GFLOP -> 1.7 GFLOP).

Distribution over 8 NeuronCores:
  - GRU tensor-parallel over the hidden dim: core c computes gate columns
    [128c:128c+128) of r,z,n for ALL 64 batches (weights pre-sharded on host).
  - An AllToAll exchanges [128-hidden-slice x 8-batch-block] tiles so each core
    ends up with rnn^T [1024, 8] for ITS 8 batches (no dynamic indexing).
  - Attention + output GEMM are batch-parallel (8 batches/core).
Matmuls run as float32r (reduced-precision fp32, 1 cyc/row, ~1.6e-4 rel err);
the context/output GEMMs use bf16 operands (their outputs tolerate ~2e-3).
"""
import numpy as np
import ml_dtypes
from contextlib import ExitStack

N, S, H = 64, 256, 1024
NCORES = 8
NB = N // NCORES        # 8 batches per core
HS = H // NCORES        # 128 hidden columns per core
KX = 2 * H // 128       # 16 contraction chunks for W_ih
KH = H // 128           # 8 contraction chunks for W_hh / W_attn
BF16 = ml_dtypes.bfloat16

_cache = {}
_last_results = None


def _build():
    import concourse.tile as tile
    from concourse import bacc, mybir

    f32 = mybir.dt.float32
    f32r = mybir.dt.float32r
    bf16 = mybir.dt.bfloat16

    nc = bacc.Bacc("TRN2", target_bir_lowering=False, debug=False,
                   enable_asserts=False, num_devices=NCORES)

    xT = nc.dram_tensor("xT", [2 * H, N], f32r, kind="ExternalInput").ap()
    hT = nc.dram_tensor("hT", [H, N], f32r, kind="ExternalInput").ap()
    hsl = nc.dram_tensor("hsl", [N, HS], f32, kind="ExternalInput").ap()
    wih = nc.dram_tensor("wih", [2 * H, 3 * HS], f32r, kind="ExternalInput").ap()
    whh = nc.dram_tensor("whh", [H, 3 * HS], f32r, kind="ExternalInput").ap()
    bi = nc.dram_tensor("bi", [1, 3 * HS], f32r, kind="ExternalInput").ap()
    bh = nc.dram_tensor("bh", [1, 3 * HS], f32r, kind="ExternalInput").ap()
    bo = nc.dram_tensor("bo", [1, H], bf16, kind="ExternalInput").ap()
    ones = nc.dram_tensor("ones", [1, N], f32r, kind="ExternalInput").ap()
    onesbf = nc.dram_tensor("onesbf", [1, NB], bf16, kind="ExternalInput").ap()
    eye = nc.dram_tensor("eye", [128, 128], f32, kind="ExternalInput").ap()
    wattn = nc.dram_tensor("wattn", [128, H], f32r, kind="ExternalInput").ap()
    woutT = nc.dram_tensor("woutT", [2 * H, H], bf16, kind="ExternalInput").ap()
    ec = nc.dram_tensor("ec", [NB, S, H], bf16, kind="ExternalInput").ap()
    et = nc.dram_tensor("et", [NB, H, S], f32r, kind="ExternalInput").ap()

    hnew_o = nc.dram_tensor("hnew_o", [N, HS], f32, kind="ExternalOutput").ap()
    ctx_o = nc.dram_tensor("ctx_o", [NB, H], f32, kind="ExternalOutput").ap()
    attn_o = nc.dram_tensor("attn_o", [NB, S], f32, kind="ExternalOutput").ap()
    out_o = nc.dram_tensor("out_o", [NB, H], f32, kind="ExternalOutput").ap()

    with tile.TileContext(nc) as tc, ExitStack() as ctx:
        sbR = ctx.enter_context(tc.tile_pool(name="sbR", bufs=1))
        sbWih = ctx.enter_context(tc.tile_pool(name="sbWih", bufs=KX))
        sbWhh = ctx.enter_context(tc.tile_pool(name="sbWhh", bufs=KH))
        sbO = ctx.enter_context(tc.tile_pool(name="sbO", bufs=8))
        sbX = ctx.enter_context(tc.tile_pool(name="sbX", bufs=2))
        sbS = ctx.enter_context(tc.tile_pool(name="sbS", bufs=1))
        sbEt = ctx.enter_context(tc.tile_pool(name="sbEt", bufs=4))
        sbEc = ctx.enter_context(tc.tile_pool(name="sbEc", bufs=NB))
        psB = ctx.enter_context(tc.tile_pool(name="psB", bufs=2, space="PSUM"))
        psS = ctx.enter_context(tc.tile_pool(name="psS", bufs=2, space="PSUM"))
        psT = ctx.enter_context(tc.tile_pool(name="psT", bufs=2, space="PSUM"))
        dram = ctx.enter_context(tc.tile_pool(name="dram", bufs=2, space="DRAM"))

        # ---- GRU weight streams issued FIRST so the collective is reached
        # early; everything later overlaps the remaining DMA stream ----
        wih_r = wih.rearrange("(k p) g -> k p g", p=128)
        whh_r = whh.rearrange("(k p) g -> k p g", p=128)
        wih_t = []
        for k in range(KX):
            w = sbWih.tile([128, 3 * HS], f32r, tag="wih")
            nc.sync.dma_start(w[:], wih_r[k])
            wih_t.append(w)
        whh_t = []
        for k in range(KH):
            w = sbWhh.tile([128, 3 * HS], f32r, tag="whh")
            nc.sync.dma_start(w[:], whh_r[k])
            whh_t.append(w)
        wattn_sb = sbR.tile([128, H], f32r)
        nc.sync.dma_start(wattn_sb[:], wattn)

        # ---- small resident loads ----
        xT_sb = sbR.tile([128, KX, N], f32r)
        nc.sync.dma_start(xT_sb[:], xT.rearrange("(k p) m -> p k m", p=128))
        hT_sb = sbR.tile([128, KH, N], f32r)
        nc.sync.dma_start(hT_sb[:], hT.rearrange("(k p) m -> p k m", p=128))
        hsl_sb = sbR.tile([N, HS], f32)
        nc.sync.dma_start(hsl_sb[:], hsl)
        eye_sb = sbR.tile([128, 128], f32)
        nc.sync.dma_start(eye_sb[:], eye)
        bi_sb = sbR.tile([1, 3 * HS], f32r)
        nc.sync.dma_start(bi_sb[:], bi)
        bh_sb = sbR.tile([1, 3 * HS], f32r)
        nc.sync.dma_start(bh_sb[:], bh)
        bo_sb = sbR.tile([1, H], bf16)
        nc.sync.dma_start(bo_sb[:], bo)
        ones_sb = sbR.tile([1, N], f32r)
        nc.sync.dma_start(ones_sb[:], ones)
        onesbf_sb = sbR.tile([1, NB], bf16)
        nc.sync.dma_start(onesbf_sb[:], onesbf)

        # ---- encoder shard loads (after weights; consumed in batch order) --
        et_tiles = []
        for n in range(NB):
            t = sbEt.tile([128, KH, S], f32r, tag="et")
            nc.sync.dma_start(t[:], et[n].rearrange("(k p) s -> p k s", p=128))
            et_tiles.append(t)
        ec_tiles = []
        for n in range(NB):
            t2 = sbEc.tile([128, 2, H], bf16, tag="ec")
            nc.sync.dma_start(t2[:], ec[n].rearrange("(sc p) h -> p sc h", p=128))
            ec_tiles.append(t2)
        woutT_r = woutT.rearrange("(k p) h -> k p h", p=128)
        wout_t = []
        for k in range(2 * KH):
            w = sbO.tile([128, H], bf16, tag="wout")
            nc.sync.dma_start(w[:], woutT_r[k])
            wout_t.append(w)

        # ---- GRU: gi/gh for this core's gate-column slice, all 64 batches --
        gi = psS.tile([N, 3 * HS], f32, tag="g")
        for k in range(KX):
            nc.tensor.matmul(gi[:], xT_sb[:, k, :], wih_t[k][:],
                             start=(k == 0), stop=False)
        nc.tensor.matmul(gi[:], ones_sb[:], bi_sb[:], start=False, stop=True)
        gh = psS.tile([N, 3 * HS], f32, tag="g")
        for k in range(KH):
            nc.tensor.matmul(gh[:], hT_sb[:, k, :], whh_t[k][:],
                             start=(k == 0), stop=False)
        nc.tensor.matmul(gh[:], ones_sb[:], bh_sb[:], start=False, stop=True)

        AF = mybir.ActivationFunctionType
        # DVE tensor_tensor can take at most one PSUM operand; stage gh in SBUF
        gh_sb = sbS.tile([N, 3 * HS], f32, tag="ghsb")
        nc.scalar.copy(gh_sb[:], gh[:])
        t1 = sbS.tile([N, HS], f32, tag="t1")
        nc.vector.tensor_add(t1[:], gi[:, 0:HS], gh_sb[:, 0:HS])
        r = sbS.tile([N, HS], f32, tag="r")
        nc.scalar.activation(r[:], t1[:], AF.Sigmoid)
        t2 = sbS.tile([N, HS], f32, tag="t2")
        nc.vector.tensor_add(t2[:], gi[:, HS:2 * HS], gh_sb[:, HS:2 * HS])
        z = sbS.tile([N, HS], f32, tag="z")
        nc.scalar.activation(z[:], t2[:], AF.Sigmoid)
        t3 = sbS.tile([N, HS], f32, tag="t3")
        nc.vector.tensor_mul(t3[:], r[:], gh_sb[:, 2 * HS:3 * HS])
        t4 = sbS.tile([N, HS], f32, tag="t4")
        nc.vector.tensor_add(t4[:], t3[:], gi[:, 2 * HS:3 * HS])
        nn_t = sbS.tile([N, HS], f32, tag="nn")
        nc.scalar.activation(nn_t[:], t4[:], AF.Tanh)
        d1 = sbS.tile([N, HS], f32, tag="d1")
        nc.vector.tensor_sub(d1[:], hsl_sb[:], nn_t[:])
        d2 = sbS.tile([N, HS], f32, tag="d2")
        nc.vector.tensor_mul(d2[:], z[:], d1[:])
        h_new = sbS.tile([N, HS], f32, tag="hn")
        nc.vector.tensor_add(h_new[:], nn_t[:], d2[:])
        nc.sync.dma_start(hnew_o, h_new[:])

        # ---- transpose h_new; partial q for ALL batches from this slice ----
        trp = psT.tile([128, N], f32, tag="tr")
        nc.tensor.transpose(trp[:], h_new[:], eye_sb[0:N, 0:N])
        hnT = sbS.tile([128, N], f32r, tag="hnT")
        nc.scalar.copy(hnT[:], trp[:])
        qp_ps = psB.tile([N, H], f32, tag="big")
        for h2 in range(2):
            nc.tensor.matmul(qp_ps[:, h2 * 512:(h2 + 1) * 512],
                             hnT[:], wattn_sb[:, h2 * 512:(h2 + 1) * 512],
                             start=True, stop=True)
        qp_sb = sbS.tile([N, H], f32, tag="qpsb")
        nc.scalar.copy(qp_sb[:], qp_ps[:])

        # ---- ONE AllToAll carries both payloads: block j = [h_newT cols
        # 8j:8j+8 (1024 el), partial-q rows 8j:8j+8 (8192 el)] ----
        PL = 128 * NB + NB * H
        g_in = dram.tile([NCORES, PL], f32)
        for j in range(NCORES):
            nc.gpsimd.dma_start(
                g_in[j, 0:128 * NB].rearrange("(p e) -> p e", p=128),
                hnT[:, j * NB:(j + 1) * NB])
            nc.sync.dma_start(
                g_in[j, 128 * NB:PL].rearrange("(p e) -> p e", p=NB),
                qp_sb[j * NB:(j + 1) * NB, :])
        g_out = dram.tile([NCORES, PL], f32)
        nc.gpsimd.collective_compute(
            "AllToAll", mybir.AluOpType.bypass,
            replica_groups=[list(range(NCORES))],
            ins=[g_in.opt()], outs=[g_out.opt()],
        )
        rnnT = sbS.tile([128, KH, NB], f32r, tag="rnnT")
        for j in range(NCORES):
            nc.gpsimd.dma_start(rnnT[:, j, :],
                              g_out[j, 0:128 * NB].rearrange("(p e) -> p e", p=128))

        # ---- sum the 8 partial-q blocks in a [64, 128] layout (all
        # partitions active); row 8*b+k of the sum is q[b, 128k:128k+128] ----
        qtmp = []
        for j in range(NCORES):
            t = sbX.tile([N, 128], f32, tag="qtmp")
            nc.sync.dma_start(t[:], g_out[j, 128 * NB:PL].rearrange("(p e) -> p e", p=N))
            qtmp.append(t)
        qs1 = []
        for j in range(4):
            a = sbX.tile([N, 128], f32, tag="qs1")
            nc.vector.tensor_add(a[:], qtmp[2 * j][:], qtmp[2 * j + 1][:])
            qs1.append(a)
        qs2 = []
        for j in range(2):
            a = sbX.tile([N, 128], f32, tag="qs2")
            nc.vector.tensor_add(a[:], qs1[2 * j][:], qs1[2 * j + 1][:])
            qs2.append(a)
        q1 = sbS.tile([N, 128], f32, tag="q1")
        nc.vector.tensor_add(q1[:], qs2[0][:], qs2[1][:])
        # relayout [64,128] (row 8b+k) -> [8,1024] (row b) with one SBUF DMA
        # so each 128-chunk transpose reads base partition 0
        q_sb = sbS.tile([NB, H], f32, tag="qsb")
        for b in range(NB):
            nc.sync.dma_start(q_sb[b:b + 1, :], q1[b * KH:(b + 1) * KH, :])
        qT = sbS.tile([128, KH, NB], f32r, tag="qT")
        for k in range(KH):
            tq = psT.tile([128, NB], f32, tag="tr")
            nc.tensor.transpose(tq[:], q_sb[:, k * 128:(k + 1) * 128],
                                eye_sb[0:NB, 0:NB])
            nc.scalar.copy(qT[:, k, :], tq[:])

        # ---- scores: per batch matvec over E^T chunks; matmul output must
        # sit at PSUM base partition 0, so each batch accumulates in [1, S]
        # and a SBUF->SBUF DMA moves it onto row n of the packed tile ----
        scores_sb = sbS.tile([NB, S], f32, tag="scb")
        for n in range(NB):
            sc_n = psT.tile([1, S], f32, tag="tr")
            for k in range(KH):
                nc.tensor.matmul(sc_n[:], qT[:, k, n:n + 1],
                                 et_tiles[n][:, k, :],
                                 start=(k == 0), stop=(k == KH - 1))
            st = sbX.tile([1, S], f32, tag="scst")
            nc.scalar.copy(st[:], sc_n[:])
            nc.sync.dma_start(scores_sb[n:n + 1, :], st[:])

        # ---- softmax over S (vectorized across the 8 batch rows) ----
        mx = sbS.tile([NB, 1], f32, tag="mx")
        nc.vector.tensor_reduce(mx[:], scores_sb[:], axis=mybir.AxisListType.X,
                                op=mybir.AluOpType.max)
        sub = sbS.tile([NB, S], f32, tag="sub")
        nc.vector.tensor_scalar_sub(sub[:], scores_sb[:], mx[:])
        ex = sbS.tile([NB, S], f32, tag="ex")
        nc.scalar.activation(ex[:], sub[:], AF.Exp)
        sm = sbS.tile([NB, 1], f32, tag="sm")
        nc.vector.tensor_reduce(sm[:], ex[:], axis=mybir.AxisListType.X,
                                op=mybir.AluOpType.add)
        rc = sbS.tile([NB, 1], f32, tag="rc")
        nc.vector.reciprocal(rc[:], sm[:])
        attn = sbS.tile([NB, S], f32, tag="attn")
        nc.vector.tensor_scalar_mul(attn[:], ex[:], rc[:])
        nc.sync.dma_start(attn_o, attn[:])
        attnT = sbS.tile([128, 2, NB], bf16, tag="attnT")
        for s2 in range(2):
            ta = psT.tile([128, NB], f32, tag="tr")
            nc.tensor.transpose(ta[:], attn[:, s2 * 128:(s2 + 1) * 128],
                                eye_sb[0:NB, 0:NB])
            nc.scalar.copy(attnT[:, s2, :], ta[:])

        # ---- context = attn @ E  [NB, H]; same base-partition-0 dance.
        # Copies alternate ACT/DVE so the 8 rows don't serialize on one
        # engine. ----
        ctx_sb = sbS.tile([NB, H], f32, tag="ctx")
        for n in range(NB):
            cx_n = psB.tile([1, H], f32, tag="big")
            for s2 in range(2):
                for h2 in range(2):
                    nc.tensor.matmul(cx_n[:, h2 * 512:(h2 + 1) * 512],
                                     attnT[:, s2, n:n + 1],
                                     ec_tiles[n][:, s2, h2 * 512:(h2 + 1) * 512],
                                     start=(s2 == 0), stop=(s2 == 1))
            cst = sbX.tile([1, H], f32, tag="cxst")
            if n % 2 == 0:
                nc.scalar.copy(cst[:], cx_n[:])
            else:
                nc.vector.tensor_copy(cst[:], cx_n[:])
            nc.sync.dma_start(ctx_sb[n:n + 1, :], cst[:])
        nc.sync.dma_start(ctx_o, ctx_sb[:])

        # ---- output = cat(rnn, context) @ W_out.T + b_out  [NB, H] ----
        rnbf = sbS.tile([128, KH, NB], bf16, tag="rnbf")
        nc.vector.tensor_copy(rnbf[:], rnnT[:])
        ctxT = sbS.tile([128, KH, NB], bf16, tag="ctxT")
        for k in range(KH):
            tcx = psT.tile([128, NB], f32, tag="tr")
            nc.tensor.transpose(tcx[:], ctx_sb[:, k * 128:(k + 1) * 128],
                                eye_sb[0:NB, 0:NB])
            nc.scalar.copy(ctxT[:, k, :], tcx[:])
        out_ps = psB.tile([NB, H], f32, tag="big")
        for k in range(2 * KH):
            lhsT = rnbf[:, k, :] if k < KH else ctxT[:, k - KH, :]
            for h2 in range(2):
                nc.tensor.matmul(out_ps[:, h2 * 512:(h2 + 1) * 512],
                                 lhsT, wout_t[k][:, h2 * 512:(h2 + 1) * 512],
                                 start=(k == 0), stop=False)
        for h2 in range(2):
            nc.tensor.matmul(out_ps[:, h2 * 512:(h2 + 1) * 512],
                             onesbf_sb[:], bo_sb[:, h2 * 512:(h2 + 1) * 512],
                             start=False, stop=True)
        out_sb = sbS.tile([NB, H], f32, tag="out")
        nc.scalar.copy(out_sb[:], out_ps[:])
        nc.sync.dma_start(out_o, out_sb[:])

    nc.compile()
    return nc


def kernel(last_output, last_context, last_hidden, encoder_outputs,
           W_ih, b_ih, W_hh, b_hh, W_attn, b_attn, W_out, b_out):
    global _last_results
    from concourse.bass_utils import run_bass_kernel_spmd

    if "nc" not in _cache:
        _cache["nc"] = _build()
    nc = _cache["nc"]

    f32 = np.float32
    h0 = np.asarray(last_hidden, f32)[0]                      # [N, H]
    x = np.concatenate([np.asarray(last_output, f32),
                        np.asarray(last_context, f32)], axis=1)
    xT = np.ascontiguousarray(x.T)                            # [2H, N]
    hT = np.ascontiguousarray(h0.T)                           # [H, N]
    enc = np.asarray(encoder_outputs, f32)                    # [N, S, H]
    W_ih = np.asarray(W_ih, f32); W_hh = np.asarray(W_hh, f32)
    W_attn = np.asarray(W_attn, f32); W_out = np.asarray(W_out, f32)
    woutT = np.ascontiguousarray(W_out.T).astype(BF16)        # [2H, H]
    eye = np.eye(128, dtype=f32)
    ones = np.ones((1, N), f32)
    onesbf = np.ones((1, NB), BF16)
    bo = np.asarray(b_out, f32).reshape(1, H).astype(BF16)

    in_maps = []
    for c in range(NCORES):
        cols = np.concatenate([g * H + np.arange(c * HS, (c + 1) * HS)
                               for g in range(3)])
        enc_c = enc[c * NB:(c + 1) * NB]
        in_maps.append({
            "xT": xT, "hT": hT,
            "hsl": np.ascontiguousarray(h0[:, c * HS:(c + 1) * HS]),
            "wih": np.ascontiguousarray(W_ih[cols, :].T),
            "whh": np.ascontiguousarray(W_hh[cols, :].T),
            "bi": np.ascontiguousarray(b_ih[cols]).reshape(1, -1).astype(f32),
            "bh": np.ascontiguousarray(b_hh[cols]).reshape(1, -1).astype(f32),
            "bo": bo, "ones": ones, "onesbf": onesbf, "eye": eye,
            "wattn": np.ascontiguousarray(W_attn[c * HS:(c + 1) * HS, :]), "woutT": woutT,
            "ec": enc_c.astype(BF16),
            "et": np.ascontiguousarray(enc_c.transpose(0, 2, 1)),
        })

    res = run_bass_kernel_spmd(nc, in_maps, core_ids=list(range(NCORES)))
    _last_results = res
    outs = res.results
    output = np.concatenate([outs[c]["out_o"] for c in range(NCORES)], axis=0)
    context = np.concatenate([outs[c]["ctx_o"] for c in range(NCORES)], axis=0)
    hidden = np.concatenate([outs[c]["hnew_o"] for c in range(NCORES)], axis=1)[None]
    attn_w = np.concatenate([outs[c]["attn_o"] for c in range(NCORES)],
                            axis=0).reshape(N, 1, S)
    return (output.astype(f32), context.astype(f32),
            hidden.astype(f32), attn_w.astype(f32))


# revision 17
# speedup vs baseline: 1.2221x; 1.1914x over previous
"""Trainium2 Bass kernel for a single-step attention decoder RNN (AttnDecoderRNN).

Math (reference semantics, N=64 batch, S=256 src len, H=1024 hidden):
  GRU step (PyTorch gate order r,z,n) -> h_new
  energy = enc @ W_attn.T + b_attn ; scores = einsum('nh,nsh->ns', h_new, energy)
  attn = softmax(scores) ; context = einsum('ns,nsh->nh', attn, enc)
  output = cat(h_new, context) @ W_out.T + b_out

Algebraic restructure: scores[n,s] = enc[n,s,:] . q[n,:] + h_new[n,:].b_attn with
q = h_new @ W_attn. The per-row constant h_new.b_attn cancels in softmax, so the
[N,S,H] energy tensor is never materialized (34 GFLOP -> 1.7 GFLOP).

Distribution over 8 NeuronCores:
  - GRU tensor-parallel over the hidden dim: core c computes gate columns
    [128c:128c+128) of r,z,n for ALL 64 batches (weights pre-sharded on host).
  - An AllToAll exchanges [128-hidden-slice x 8-batch-block] tiles so each core
    ends up with rnn^T [1024, 8] for ITS 8 batches (no dynamic indexing).
  - Attention + output GEMM are batch-parallel (8 batches/core).
Matmuls run as float32r (reduced-precision fp32, 1 cyc/row, ~1.6e-4 rel err);
the context/output GEMMs use bf16 operands (their outputs tolerate ~2e-3).
"""
import numpy as np
import ml_dtypes
from contextlib import ExitStack

N, S, H = 64, 256, 1024
NCORES = 8
NB = N // NCORES        # 8 batches per core
HS = H // NCORES        # 128 hidden columns per core
KX = 2 * H // 128       # 16 contraction chunks for W_ih
KH = H // 128           # 8 contraction chunks for W_hh / W_attn
BF16 = ml_dtypes.bfloat16

_cache = {}
_last_results = None


def _build():
    import concourse.tile as tile
    from concourse import bacc, mybir

    f32 = mybir.dt.float32
    f32r = mybir.dt.float32r
    bf16 = mybir.dt.bfloat16

    nc = bacc.Bacc("TRN2", target_bir_lowering=False, debug=False,
                   enable_asserts=False, num_devices=NCORES)

    xT = nc.dram_tensor("xT", [2 * H, N], f32r, kind="ExternalInput").ap()
    hT = nc.dram_tensor("hT", [H, N], f32r, kind="ExternalInput").ap()
    hsl = nc.dram_tensor("hsl", [N, HS], f32, kind="ExternalInput").ap()
    wih = nc.dram_tensor("wih", [2 * H, 3 * HS], f32r, kind="ExternalInput").ap()
    whh = nc.dram_tensor("whh", [H, 3 * HS], f32r, kind="ExternalInput").ap()
    bi = nc.dram_tensor("bi", [1, 3 * HS], f32r, kind="ExternalInput").ap()
    bh = nc.dram_tensor("bh", [1, 3 * HS], f32r, kind="ExternalInput").ap()
    bo = nc.dram_tensor("bo", [1, H], bf16, kind="ExternalInput").ap()
    ones = nc.dram_tensor("ones", [1, N], f32r, kind="ExternalInput").ap()
    onesbf = nc.dram_tensor("onesbf", [1, NB], bf16, kind="ExternalInput").ap()
    eye = nc.dram_tensor("eye", [128, 128], f32, kind="ExternalInput").ap()
    wattn = nc.dram_tensor("wattn", [128, H], f32r, kind="ExternalInput").ap()
    woutT = nc.dram_tensor("woutT", [2 * H, H], bf16, kind="ExternalInput").ap()
    ec = nc.dram_tensor("ec", [NB, S, H], bf16, kind="ExternalInput").ap()
    et = nc.dram_tensor("et", [NB, H, S], f32r, kind="ExternalInput").ap()

    hnew_o = nc.dram_tensor("hnew_o", [N, HS], f32, kind="ExternalOutput").ap()
    ctx_o = nc.dram_tensor("ctx_o", [NB, H], f32, kind="ExternalOutput").ap()
    attn_o = nc.dram_tensor("attn_o", [NB, S], f32, kind="ExternalOutput").ap()
    out_o = nc.dram_tensor("out_o", [NB, H], f32, kind="ExternalOutput").ap()

    with tile.TileContext(nc) as tc, ExitStack() as ctx:
        sbR = ctx.enter_context(tc.tile_pool(name="sbR", bufs=1))
        sbWih = ctx.enter_context(tc.tile_pool(name="sbWih", bufs=12))
        sbWhh = ctx.enter_context(tc.tile_pool(name="sbWhh", bufs=KH))
        sbO = ctx.enter_context(tc.tile_pool(name="sbO", bufs=6))
        sbX = ctx.enter_context(tc.tile_pool(name="sbX", bufs=2))
        sbS = ctx.enter_context(tc.tile_pool(name="sbS", bufs=1))
        sbEt = ctx.enter_context(tc.tile_pool(name="sbEt", bufs=NB))
        sbEc = ctx.enter_context(tc.tile_pool(name="sbEc", bufs=NB))
        psB = ctx.enter_context(tc.tile_pool(name="psB", bufs=2, space="PSUM"))
        psS = ctx.enter_context(tc.tile_pool(name="psS", bufs=2, space="PSUM"))
        psT = ctx.enter_context(tc.tile_pool(name="psT", bufs=2, space="PSUM"))
        dram = ctx.enter_context(tc.tile_pool(name="dram", bufs=2, space="DRAM"))

        # ---- GRU weight streams issued FIRST so the collective is reached
        # early; everything later overlaps the remaining DMA stream ----
        xT_sb = sbR.tile([128, KX, N], f32r)
        nc.sync.dma_start(xT_sb[:], xT.rearrange("(k p) m -> p k m", p=128))
        hT_sb = sbR.tile([128, KH, N], f32r)
        nc.sync.dma_start(hT_sb[:], hT.rearrange("(k p) m -> p k m", p=128))

        wih_r = wih.rearrange("(k p) g -> k p g", p=128)
        whh_r = whh.rearrange("(k p) g -> k p g", p=128)
        wih_t = []
        for k in range(KX):
            w = sbWih.tile([128, 3 * HS], f32r, tag="wih")
            nc.sync.dma_start(w[:], wih_r[k])
            wih_t.append(w)
        whh_t = []
        for k in range(KH):
            w = sbWhh.tile([128, 3 * HS], f32r, tag="whh")
            nc.sync.dma_start(w[:], whh_r[k])
            whh_t.append(w)
        wattn_sb = sbR.tile([128, H], f32r)
        nc.sync.dma_start(wattn_sb[:], wattn)

        # ---- small resident loads ----
        hsl_sb = sbR.tile([N, HS], f32)
        nc.sync.dma_start(hsl_sb[:], hsl)
        eye_sb = sbR.tile([128, 128], f32)
        nc.sync.dma_start(eye_sb[:], eye)
        bi_sb = sbR.tile([1, 3 * HS], f32r)
        nc.sync.dma_start(bi_sb[:], bi)
        bh_sb = sbR.tile([1, 3 * HS], f32r)
        nc.sync.dma_start(bh_sb[:], bh)
        bo_sb = sbR.tile([1, H], bf16)
        nc.sync.dma_start(bo_sb[:], bo)
        ones_sb = sbR.tile([1, N], f32r)
        nc.sync.dma_start(ones_sb[:], ones)
        onesbf_sb = sbR.tile([1, NB], bf16)
        nc.sync.dma_start(onesbf_sb[:], onesbf)

        # ---- encoder shard loads (after weights; consumed in batch order) --
        et_tiles = []
        for n in range(NB):
            t = sbEt.tile([128, KH, S], f32r, tag="et")
            nc.sync.dma_start(t[:], et[n].rearrange("(k p) s -> p k s", p=128))
            et_tiles.append(t)
        ec_tiles = []
        for n in range(NB):
            t2 = sbEc.tile([128, 2, H], bf16, tag="ec")
            nc.sync.dma_start(t2[:], ec[n].rearrange("(sc p) h -> p sc h", p=128))
            ec_tiles.append(t2)
        woutT_r = woutT.rearrange("(k p) h -> k p h", p=128)
        wout_t = []
        for k in range(2 * KH):
            w = sbO.tile([128, H], bf16, tag="wout")
            nc.sync.dma_start(w[:], woutT_r[k])
            wout_t.append(w)

        # ---- GRU: gi/gh for this core's gate-column slice, all 64 batches --
        gi = psS.tile([N, 3 * HS], f32, tag="g")
        for k in range(KX):
            nc.tensor.matmul(gi[:], xT_sb[:, k, :], wih_t[k][:],
                             start=(k == 0), stop=False)
        nc.tensor.matmul(gi[:], ones_sb[:], bi_sb[:], start=False, stop=True)
        gh = psS.tile([N, 3 * HS], f32, tag="g")
        for k in range(KH):
            nc.tensor.matmul(gh[:], hT_sb[:, k, :], whh_t[k][:],
                             start=(k == 0), stop=False)
        nc.tensor.matmul(gh[:], ones_sb[:], bh_sb[:], start=False, stop=True)

        AF = mybir.ActivationFunctionType
        # DVE tensor_tensor can take at most one PSUM operand; stage gh in SBUF
        gh_sb = sbS.tile([N, 3 * HS], f32, tag="ghsb")
        nc.scalar.copy(gh_sb[:], gh[:])
        t1 = sbS.tile([N, HS], f32, tag="t1")
        nc.vector.tensor_add(t1[:], gi[:, 0:HS], gh_sb[:, 0:HS])
        r = sbS.tile([N, HS], f32, tag="r")
        nc.scalar.activation(r[:], t1[:], AF.Sigmoid)
        t2 = sbS.tile([N, HS], f32, tag="t2")
        nc.vector.tensor_add(t2[:], gi[:, HS:2 * HS], gh_sb[:, HS:2 * HS])
        z = sbS.tile([N, HS], f32, tag="z")
        nc.scalar.activation(z[:], t2[:], AF.Sigmoid)
        t3 = sbS.tile([N, HS], f32, tag="t3")
        nc.vector.tensor_mul(t3[:], r[:], gh_sb[:, 2 * HS:3 * HS])
        t4 = sbS.tile([N, HS], f32, tag="t4")
        nc.vector.tensor_add(t4[:], t3[:], gi[:, 2 * HS:3 * HS])
        nn_t = sbS.tile([N, HS], f32, tag="nn")
        nc.scalar.activation(nn_t[:], t4[:], AF.Tanh)
        d1 = sbS.tile([N, HS], f32, tag="d1")
        nc.vector.tensor_sub(d1[:], hsl_sb[:], nn_t[:])
        d2 = sbS.tile([N, HS], f32, tag="d2")
        nc.vector.tensor_mul(d2[:], z[:], d1[:])
        h_new = sbS.tile([N, HS], f32, tag="hn")
        nc.vector.tensor_add(h_new[:], nn_t[:], d2[:])
        nc.scalar.dma_start(hnew_o, h_new[:])

        # ---- transpose h_new; partial q for ALL batches from this slice ----
        trp = psT.tile([128, N], f32, tag="tr")
        nc.tensor.transpose(trp[:], h_new[:], eye_sb[0:N, 0:N])
        hnT = sbS.tile([128, N], f32r, tag="hnT")
        nc.scalar.copy(hnT[:], trp[:])
        qp_ps = psB.tile([N, H], f32, tag="big")
        for h2 in range(2):
            nc.tensor.matmul(qp_ps[:, h2 * 512:(h2 + 1) * 512],
                             hnT[:], wattn_sb[:, h2 * 512:(h2 + 1) * 512],
                             start=True, stop=True)
        qp_sb = sbS.tile([N, H], f32, tag="qpsb")
        nc.scalar.copy(qp_sb[:], qp_ps[:])

        # ---- ONE AllToAll carries both payloads: block j = [h_newT cols
        # 8j:8j+8 (1024 el), partial-q rows 8j:8j+8 (8192 el)] ----
        PL = 128 * NB + NB * H
        g_in = dram.tile([NCORES, PL], f32)
        for j in range(NCORES):
            nc.gpsimd.dma_start(
                g_in[j, 0:128 * NB].rearrange("(p e) -> p e", p=128),
                hnT[:, j * NB:(j + 1) * NB])
            nc.scalar.dma_start(
                g_in[j, 128 * NB:PL].rearrange("(p e) -> p e", p=NB),
                qp_sb[j * NB:(j + 1) * NB, :])
        g_out = dram.tile([NCORES, PL], f32)
        nc.gpsimd.collective_compute(
            "AllToAll", mybir.AluOpType.bypass,
            replica_groups=[list(range(NCORES))],
            ins=[g_in.opt()], outs=[g_out.opt()],
        )
        rnnT = sbS.tile([128, KH, NB], f32r, tag="rnnT")
        for j in range(NCORES):
            nc.gpsimd.dma_start(rnnT[:, j, :],
                              g_out[j, 0:128 * NB].rearrange("(p e) -> p e", p=128))

        # ---- sum the 8 partial-q blocks in a [64, 128] layout (all
        # partitions active); row 8*b+k of the sum is q[b, 128k:128k+128] ----
        qtmp = []
        for j in range(NCORES):
            t = sbX.tile([N, 128], f32, tag="qtmp")
            nc.scalar.dma_start(t[:], g_out[j, 128 * NB:PL].rearrange("(p e) -> p e", p=N))
            qtmp.append(t)
        qs1 = []
        for j in range(4):
            a = sbX.tile([N, 128], f32, tag="qs1")
            nc.vector.tensor_add(a[:], qtmp[2 * j][:], qtmp[2 * j + 1][:])
            qs1.append(a)
        qs2 = []
        for j in range(2):
            a = sbX.tile([N, 128], f32, tag="qs2")
            nc.vector.tensor_add(a[:], qs1[2 * j][:], qs1[2 * j + 1][:])
            qs2.append(a)
        q1 = sbS.tile([N, 128], f32, tag="q1")
        nc.vector.tensor_add(q1[:], qs2[0][:], qs2[1][:])
        # relayout [64,128] (row 8b+k) -> [8,1024] (row b) with one SBUF DMA
        # so each 128-chunk transpose reads base partition 0
        q_sb = sbS.tile([NB, H], f32, tag="qsb")
        for b in range(NB):
            nc.scalar.dma_start(q_sb[b:b + 1, :], q1[b * KH:(b + 1) * KH, :])
        qT = sbS.tile([128, KH, NB], f32r, tag="qT")
        for k in range(KH):
            tq = psT.tile([128, NB], f32, tag="tr")
            nc.tensor.transpose(tq[:], q_sb[:, k * 128:(k + 1) * 128],
                                eye_sb[0:NB, 0:NB])
            nc.scalar.copy(qT[:, k, :], tq[:])

        # ---- scores: per batch matvec over E^T chunks; matmul output must
        # sit at PSUM base partition 0, so each batch accumulates in [1, S]
        # and a SBUF->SBUF DMA moves it onto row n of the packed tile ----
        scores_sb = sbS.tile([NB, S], f32, tag="scb")
        for n in range(NB):
            sc_n = psT.tile([1, S], f32, tag="tr")
            for k in range(KH):
                nc.tensor.matmul(sc_n[:], qT[:, k, n:n + 1],
                                 et_tiles[n][:, k, :],
                                 start=(k == 0), stop=(k == KH - 1))
            st = sbX.tile([1, S], f32, tag="scst")
            nc.scalar.copy(st[:], sc_n[:])
            nc.scalar.dma_start(scores_sb[n:n + 1, :], st[:])

        # ---- softmax over S (vectorized across the 8 batch rows) ----
        mx = sbS.tile([NB, 1], f32, tag="mx")
        nc.vector.tensor_reduce(mx[:], scores_sb[:], axis=mybir.AxisListType.X,
                                op=mybir.AluOpType.max)
        sub = sbS.tile([NB, S], f32, tag="sub")
        nc.vector.tensor_scalar_sub(sub[:], scores_sb[:], mx[:])
        ex = sbS.tile([NB, S], f32, tag="ex")
        nc.scalar.activation(ex[:], sub[:], AF.Exp)
        sm = sbS.tile([NB, 1], f32, tag="sm")
        nc.vector.tensor_reduce(sm[:], ex[:], axis=mybir.AxisListType.X,
                                op=mybir.AluOpType.add)
        rc = sbS.tile([NB, 1], f32, tag="rc")
        nc.vector.reciprocal(rc[:], sm[:])
        attn = sbS.tile([NB, S], f32, tag="attn")
        nc.vector.tensor_scalar_mul(attn[:], ex[:], rc[:])
        nc.scalar.dma_start(attn_o, attn[:])
        attnT = sbS.tile([128, 2, NB], bf16, tag="attnT")
        for s2 in range(2):
            ta = psT.tile([128, NB], f32, tag="tr")
            nc.tensor.transpose(ta[:], attn[:, s2 * 128:(s2 + 1) * 128],
                                eye_sb[0:NB, 0:NB])
            nc.scalar.copy(attnT[:, s2, :], ta[:])

        # ---- context = attn @ E  [NB, H]; same base-partition-0 dance.
        # Copies alternate ACT/DVE so the 8 rows don't serialize on one
        # engine. ----
        ctx_sb = sbS.tile([NB, H], f32, tag="ctx")
        for n in range(NB):
            cx_n = psB.tile([1, H], f32, tag="big")
            for s2 in range(2):
                for h2 in range(2):
                    nc.tensor.matmul(cx_n[:, h2 * 512:(h2 + 1) * 512],
                                     attnT[:, s2, n:n + 1],
                                     ec_tiles[n][:, s2, h2 * 512:(h2 + 1) * 512],
                                     start=(s2 == 0), stop=(s2 == 1))
            cst = sbX.tile([1, H], f32, tag="cxst")
            if n % 2 == 0:
                nc.scalar.copy(cst[:], cx_n[:])
            else:
                nc.vector.tensor_copy(cst[:], cx_n[:])
            nc.scalar.dma_start(ctx_sb[n:n + 1, :], cst[:])
        nc.scalar.dma_start(ctx_o, ctx_sb[:])

        # ---- output = cat(rnn, context) @ W_out.T + b_out  [NB, H] ----
        rnbf = sbS.tile([128, KH, NB], bf16, tag="rnbf")
        nc.vector.tensor_copy(rnbf[:], rnnT[:])
        ctxT = sbS.tile([128, KH, NB], bf16, tag="ctxT")
        for k in range(KH):
            tcx = psT.tile([128, NB], f32, tag="tr")
            nc.tensor.transpose(tcx[:], ctx_sb[:, k * 128:(k + 1) * 128],
                                eye_sb[0:NB, 0:NB])
            nc.scalar.copy(ctxT[:, k, :], tcx[:])
        out_ps = psB.tile([NB, H], f32, tag="big")
        for k in range(2 * KH):
            lhsT = rnbf[:, k, :] if k < KH else ctxT[:, k - KH, :]
            for h2 in range(2):
                nc.tensor.matmul(out_ps[:, h2 * 512:(h2 + 1) * 512],
                                 lhsT, wout_t[k][:, h2 * 512:(h2 + 1) * 512],
                                 start=(k == 0), stop=False)
        for h2 in range(2):
            nc.tensor.matmul(out_ps[:, h2 * 512:(h2 + 1) * 512],
                             onesbf_sb[:], bo_sb[:, h2 * 512:(h2 + 1) * 512],
                             start=False, stop=True)
        out_sb = sbS.tile([NB, H], f32, tag="out")
        nc.scalar.copy(out_sb[:], out_ps[:])
        nc.scalar.dma_start(out_o, out_sb[:])

    nc.compile()
    return nc


def kernel(last_output, last_context, last_hidden, encoder_outputs,
           W_ih, b_ih, W_hh, b_hh, W_attn, b_attn, W_out, b_out):
    global _last_results
    from concourse.bass_utils import run_bass_kernel_spmd

    if "nc" not in _cache:
        _cache["nc"] = _build()
    nc = _cache["nc"]

    f32 = np.float32
    h0 = np.asarray(last_hidden, f32)[0]                      # [N, H]
    x = np.concatenate([np.asarray(last_output, f32),
                        np.asarray(last_context, f32)], axis=1)
    xT = np.ascontiguousarray(x.T)                            # [2H, N]
    hT = np.ascontiguousarray(h0.T)                           # [H, N]
    enc = np.asarray(encoder_outputs, f32)                    # [N, S, H]
    W_ih = np.asarray(W_ih, f32); W_hh = np.asarray(W_hh, f32)
    W_attn = np.asarray(W_attn, f32); W_out = np.asarray(W_out, f32)
    woutT = np.ascontiguousarray(W_out.T).astype(BF16)        # [2H, H]
    eye = np.eye(128, dtype=f32)
    ones = np.ones((1, N), f32)
    onesbf = np.ones((1, NB), BF16)
    bo = np.asarray(b_out, f32).reshape(1, H).astype(BF16)

    in_maps = []
    for c in range(NCORES):
        cols = np.concatenate([g * H + np.arange(c * HS, (c + 1) * HS)
                               for g in range(3)])
        enc_c = enc[c * NB:(c + 1) * NB]
        in_maps.append({
            "xT": xT, "hT": hT,
            "hsl": np.ascontiguousarray(h0[:, c * HS:(c + 1) * HS]),
            "wih": np.ascontiguousarray(W_ih[cols, :].T),
            "whh": np.ascontiguousarray(W_hh[cols, :].T),
            "bi": np.ascontiguousarray(b_ih[cols]).reshape(1, -1).astype(f32),
            "bh": np.ascontiguousarray(b_hh[cols]).reshape(1, -1).astype(f32),
            "bo": bo, "ones": ones, "onesbf": onesbf, "eye": eye,
            "wattn": np.ascontiguousarray(W_attn[c * HS:(c + 1) * HS, :]), "woutT": woutT,
            "ec": enc_c.astype(BF16),
            "et": np.ascontiguousarray(enc_c.transpose(0, 2, 1)),
        })

    res = run_bass_kernel_spmd(nc, in_maps, core_ids=list(range(NCORES)))
    _last_results = res
    outs = res.results
    output = np.concatenate([outs[c]["out_o"] for c in range(NCORES)], axis=0)
    context = np.concatenate([outs[c]["ctx_o"] for c in range(NCORES)], axis=0)
    hidden = np.concatenate([outs[c]["hnew_o"] for c in range(NCORES)], axis=1)[None]
    attn_w = np.concatenate([outs[c]["attn_o"] for c in range(NCORES)],
                            axis=0).reshape(N, 1, S)
    return (output.astype(f32), context.astype(f32),
            hidden.astype(f32), attn_w.astype(f32))
